# revision 19
# baseline (speedup 1.0000x reference)
"""Trainium2 Bass kernel for nn_CorefMergeLayer.

Reference semantics (per example b):
    cl = m_bank[coref_posi[b], b, :]            # [K, C, H] gathered mentions
    q = cl @ Wq ; k = cl @ Uk
    scores[k,i,j] = v . tanh(q_i + k_j + b_attn)
    alpha = softmax_j(scores)
    ctx = alpha @ cl
    attn_h = [ctx; cl] @ W_out + b_out
    mrg = tanh([cl; attn_h] @ W_mrg + b_mrg)
    out = m_bank with mention rows replaced by mrg

Sharding: data-parallel over batch B=16 across 8 cores (BL=2 examples per
core); weights replicated; W_out/W_mrg/Wq/Uk streamed from HBM.
"""

import sys

for _p in ("/opt/trn_rl_repo",):
    if _p not in sys.path:
        sys.path.insert(0, _p)

import numpy as np

import concourse.bacc as bacc
import concourse.bass as bass
import concourse.mybir as mybir
import concourse.tile as tile
from concourse.bass import IndirectOffsetOnAxis
from concourse.masks import make_identity
import bass_rust as _bass_rust


def _vec_pairs(dims):
    return _bass_rust.VecI64Pair([list(d) for d in dims])


F32 = mybir.dt.float32
I32 = mybir.dt.int32
AF = mybir.ActivationFunctionType
ALU = mybir.AluOpType
AX = mybir.AxisListType

P = 128  # partitions


def build_program(S=1024, BL=2, H=1024, K=8, C=16, wdt=F32):
    """Build the SPMD per-core Bass program.

    Per-core inputs:
      mb    [S, BL, H] f32   batch slice of m_bank
      idx   [MT, BL]   i32   row indices into the (S*BL, H) view of mb
      Wq,Uk [H, H]     f32
      vattn [H]        f32
      battn,bout,bmrg [1, H] f32
      W_out,W_mrg [2H, H] f32
    Output:
      out   [S, BL, H] f32
    """
    MT = K * C                 # mentions per example (<= 128)
    M2 = BL * MT               # mention columns, both examples
    NB = H // P                # h blocks
    PAIRS_E = K * C * C        # pair columns per example
    assert MT <= P and H % P == 0 and M2 <= 512

    nc = bacc.Bacc()

    mb = nc.dram_tensor("mb", [S, BL, H], F32, kind="ExternalInput")
    idx = nc.dram_tensor("idx", [MT, BL], I32, kind="ExternalInput")
    Wq = nc.dram_tensor("Wq", [H, H], wdt, kind="ExternalInput")
    Uk = nc.dram_tensor("Uk", [H, H], wdt, kind="ExternalInput")
    vattn = nc.dram_tensor("vattn", [H], wdt, kind="ExternalInput")
    battn = nc.dram_tensor("battn", [1, H], wdt, kind="ExternalInput")
    Wout = nc.dram_tensor("Wout", [2 * H, H], wdt, kind="ExternalInput")
    bout = nc.dram_tensor("bout", [1, H], wdt, kind="ExternalInput")
    Wmrg = nc.dram_tensor("Wmrg", [2 * H, H], wdt, kind="ExternalInput")
    bmrg = nc.dram_tensor("bmrg", [1, H], wdt, kind="ExternalInput")
    out = nc.dram_tensor("out", [S, BL, H], F32, kind="ExternalOutput")

    mb2d = mb[:, :, :].rearrange("s b h -> (s b) h")
    out2d = out[:, :, :].rearrange("s b h -> (s b) h")

    with tile.TileContext(nc) as tc:
        with tc.tile_pool(name="persist", bufs=1) as pp:
            # ---- passthrough copy mb -> out (DRAM->DRAM), issued first so
            # it overlaps all compute; the final scatter depends on it (WAW).
            ncopy = 2
            rows = S // ncopy
            for i in range(ncopy):
                nc.scalar.dma_start(
                    out=out[i * rows:(i + 1) * rows, :, :],
                    in_=mb[i * rows:(i + 1) * rows, :, :],
                )

            # ---- constants into SBUF
            idx_sb = pp.tile([MT, BL], I32, tag="idx", name="idx_sb")
            nc.sync.dma_start(out=idx_sb[:, :], in_=idx[:, :])

            vT_sb = pp.tile([P, NB], wdt, tag="vT", name="vT_sb")  # vT[p, nb] = v[nb*128+p]
            nc.sync.dma_start(
                out=vT_sb[:, :], in_=vattn[:].rearrange("(nb p) -> p nb", p=P)
            )

            battn_sb = pp.tile([1, H], wdt, tag="battn", name="battn_sb")
            nc.sync.dma_start(out=battn_sb[:, :], in_=battn[:, :])
            bout_sb = pp.tile([1, H], wdt, tag="bout", name="bout_sb")
            nc.sync.dma_start(out=bout_sb[:, :], in_=bout[:, :])
            bmrg_sb = pp.tile([1, H], wdt, tag="bmrg", name="bmrg_sb")
            nc.sync.dma_start(out=bmrg_sb[:, :], in_=bmrg[:, :])

            ones_sb = pp.tile([1, max(M2, P)], wdt, tag="ones", name="ones_sb")
            nc.gpsimd.memset(ones_sb[:, :], 1.0)

            ident = pp.tile([P, P], F32, tag="ident", name="ident")
            make_identity(nc, ident[:, :])

            # row image of the block-diagonal alphaT matrix (partition 0);
            # zeroed once early -- only the diagonal blocks are ever rewritten.
            # Shared across examples (sequential use) to save SBUF.
            diag = pp.tile([1, MT * MT], F32, tag="diag", name="diag")
            nc.gpsimd.memset(diag[:, :], 0.0)

            # ---- gather mentions: cl[e] [MT, H] mention-major
            cl_sb = [pp.tile([MT, H], F32, tag=f"cl{e}", name=f"cl{e}") for e in range(BL)]
            for e in range(BL):
                nc.gpsimd.indirect_dma_start(
                    out=cl_sb[e][:, :],
                    out_offset=None,
                    in_=mb2d,
                    in_offset=IndirectOffsetOnAxis(ap=idx_sb[:, e:e + 1], axis=0),
                )

            # ---- transpose to clT blocks [128, M2]
            clT = [pp.tile([P, M2], wdt, tag=f"clT{b}", name=f"clT{b}") for b in range(NB)]
            with tc.tile_pool(name="tp_psum", bufs=3, space="PSUM") as tpp:
                for e in range(BL):
                    for b in range(NB):
                        tp = tpp.tile([P, MT], F32, tag="tp", name="tp")
                        nc.tensor.transpose(
                            out=tp[:, :],
                            in_=cl_sb[e][:, b * P:(b + 1) * P],
                            identity=ident[:MT, :MT],
                        )
                        nc.vector.tensor_copy(
                            out=clT[b][:, e * MT:(e + 1) * MT], in_=tp[:, :]
                        )

            # ---- projections: QT/KT blocks [128, M2];  KT += b_attn
            QT = [pp.tile([P, M2], wdt, tag=f"QT{b}", name=f"QT{b}") for b in range(NB)]
            KT = [pp.tile([P, M2], wdt, tag=f"KT{b}", name=f"KT{b}") for b in range(NB)]
            with tc.tile_pool(name="wcol", bufs=2) as wp, \
                 tc.tile_pool(name="qk_psum", bufs=2, space="PSUM") as qkp:
                # paired column loads: one DMA covers two ho blocks
                # wq_col[p, hi*2P + c] = Wq[hi*128 + p, ho2*2P + c], c in [0, 2P)
                wq_col = uk_col = None
                for ho in range(NB):
                    if ho % 2 == 0:
                        ho2 = ho // 2
                        wq_col = wp.tile([P, 2 * H], wdt, tag="wq", name="wq")
                        nc.scalar.dma_start(
                            out=wq_col[:, :].rearrange("p (hi c) -> p hi c", hi=NB),
                            in_=Wq[:, ho2 * 2 * P:(ho2 + 1) * 2 * P].rearrange(
                                "(hi p) c -> p hi c", p=P
                            ),
                        )
                        uk_col = wp.tile([P, 2 * H], wdt, tag="uk", name="uk")
                        nc.scalar.dma_start(
                            out=uk_col[:, :].rearrange("p (hi c) -> p hi c", hi=NB),
                            in_=Uk[:, ho2 * 2 * P:(ho2 + 1) * 2 * P].rearrange(
                                "(hi p) c -> p hi c", p=P
                            ),
                        )
                    off = (ho % 2) * P
                    qt_p = qkp.tile([P, M2], F32, tag="qt", name="qt")
                    kt_p = qkp.tile([P, M2], F32, tag="kt", name="kt")
                    for hi in range(NB):
                        nc.tensor.matmul(
                            out=qt_p[:, :],
                            lhsT=wq_col[:, hi * 2 * P + off:hi * 2 * P + off + P],
                            rhs=clT[hi][:, :],
                            start=(hi == 0), stop=(hi == NB - 1),
                        )
                    # KT = b_attn (rank-1: b_chunk^T @ ones), then += Uk.T cl
                    nc.tensor.matmul(
                        out=kt_p[:, :],
                        lhsT=battn_sb[0:1, ho * P:(ho + 1) * P],
                        rhs=ones_sb[0:1, :M2],
                        start=True, stop=False,
                    )
                    for hi in range(NB):
                        nc.tensor.matmul(
                            out=kt_p[:, :],
                            lhsT=uk_col[:, hi * 2 * P + off:hi * 2 * P + off + P],
                            rhs=clT[hi][:, :],
                            start=False, stop=(hi == NB - 1),
                        )
                    nc.vector.tensor_copy(out=QT[ho][:, :], in_=qt_p[:, :])
                    nc.scalar.activation(out=KT[ho][:, :], in_=kt_p[:, :], func=AF.Copy)

            # ---- pair scores: sc[e][(k,j,i)] = sum_h v[h]*tanh(q_i+k_j+b)
            NCH = (PAIRS_E + 511) // 512  # 512-col psum chunks per example
            with tc.tile_pool(name="sc_psum", bufs=1, space="PSUM") as scp, \
                 tc.tile_pool(name="epool", bufs=2) as ep:
                sc_p = [scp.tile([1, PAIRS_E], F32, tag=f"sc{e}", name=f"sc{e}") for e in range(BL)]
                for hb in range(NB):
                    for e in range(BL):
                        e_in = ep.tile([P, PAIRS_E], wdt, tag="e_in", name="e_in")
                        kin = KT[hb][:, e * MT:(e + 1) * MT] \
                            .rearrange("p (k j) -> p k j", k=K) \
                            .unsqueeze(3).to_broadcast([P, K, C, C])
                        qin = QT[hb][:, e * MT:(e + 1) * MT] \
                            .rearrange("p (k i) -> p k i", k=K) \
                            .unsqueeze(2).to_broadcast([P, K, C, C])
                        eng = nc.vector if e == 0 else nc.gpsimd
                        eng.tensor_tensor(
                            out=e_in[:, :].rearrange("p (k j i) -> p k j i", k=K, j=C),
                            in0=kin, in1=qin, op=ALU.add,
                        )
                        e_t = ep.tile([P, PAIRS_E], wdt, tag="e_t", name="e_t", bufs=1)
                        nc.scalar.activation(out=e_t[:, :], in_=e_in[:, :], func=AF.Tanh)
                        for ch in range(NCH):
                            c0, c1 = ch * 512, min((ch + 1) * 512, PAIRS_E)
                            nc.tensor.matmul(
                                out=sc_p[e][0:1, c0:c1],
                                lhsT=vT_sb[:, hb:hb + 1],
                                rhs=e_t[:, c0:c1],
                                start=(hb == 0), stop=(hb == NB - 1),
                            )

                # ---- softmax over j; alpha laid out (k, j, i)
                abd = [pp.tile([MT, MT], F32, tag=f"abd{e}", name=f"abd{e}") for e in range(BL)]
                for e in range(BL):
                    # exp written strided into the pre-zeroed row image of the
                    # block-diagonal alphaT matrix: position of value (k,j,i)
                    # is (k*C+j)*MT + k*C + i (affine in (k,j,i)).  After the
                    # in-place normalize, one DMA reshapes the row image to
                    # the [MT, MT] tile, so the ctx matmul depends on a
                    # single DMA (HW sync-wait limit).
                    p0 = list(list(diag[0:1, :].ap)[0])
                    dg_kji = diag[0:1, :].copy()
                    dg_kji.ap = _vec_pairs([p0, [C * MT + C, K], [MT, C], [1, C]])
                    dg_kij = diag[0:1, :].copy()
                    dg_kij.ap = _vec_pairs([p0, [C * MT + C, K], [1, C], [MT, C]])
                    nc.scalar.activation(
                        out=dg_kji, in_=sc_p[e][0:1, :].rearrange(
                            "p (k j i) -> p k j i", k=K, j=C),
                        func=AF.Exp,
                    )
                    sum_sb = pp.tile([1, K * C], F32, tag=f"sum{e}", name=f"sumsb{e}")
                    nc.vector.tensor_reduce(
                        out=sum_sb[:, :], in_=dg_kij, axis=AX.X, op=ALU.add,
                    )
                    rs_sb = pp.tile([1, K * C], F32, tag=f"rs{e}", name=f"rssb{e}")
                    nc.vector.reciprocal(out=rs_sb[:, :], in_=sum_sb[:, :])
                    nc.vector.tensor_tensor(
                        out=dg_kji, in0=dg_kji,
                        in1=rs_sb[:, :].rearrange("p (k i) -> p k i", k=K)
                            .unsqueeze(2).to_broadcast([1, K, C, C]),
                        op=ALU.mult,
                    )
                    nc.sync.dma_start(out=abd[e][:, :], in_=diag[0:1, :])

            # ---- ctx: ctxT[h, (k,i)] = sum_(k,j) cl[(k,j), h] * abd[(k,j),(k,i)]
            ctxT = [pp.tile([P, M2], wdt, tag=f"ctxT{b}", name=f"ctxT{b}") for b in range(NB)]
            with tc.tile_pool(name="ctx_psum", bufs=3, space="PSUM") as cxp:
                for e in range(BL):
                    for b in range(NB):
                        cx = cxp.tile([P, MT], F32, tag="cx", name="cx")
                        nc.tensor.matmul(
                            out=cx[:, :],
                            lhsT=cl_sb[e][:, b * P:(b + 1) * P],
                            rhs=abd[e][:, :],
                            start=True, stop=True,
                        )
                        nc.vector.tensor_copy(
                            out=ctxT[b][:, e * MT:(e + 1) * MT], in_=cx[:, :]
                        )

            # ---- attn_h: attnT[d, m] = sum_f W_out[f, d] * cat1T[f, m] + b_out[d]
            # cat1T blocks: f 0..NB-1 -> ctxT, NB..2NB-1 -> clT
            attnT = [pp.tile([P, M2], wdt, tag=f"attnT{b}", name=f"attnT{b}") for b in range(NB)]
            with tc.tile_pool(name="wrow", bufs=2) as wrp, \
                 tc.tile_pool(name="at_psum", bufs=1, space="PSUM") as atp:
                # two d-blocks per PSUM bank tile -> 4 banks, leaving room for
                # the mrg-phase PSUM so its clT half overlaps this phase
                at_p = [atp.tile([P, 2 * M2], F32, tag=f"at{d}", name=f"at{d}")
                        for d in range(NB // 2)]

                def at_slice(d):
                    return at_p[d // 2][:, (d % 2) * M2:(d % 2 + 1) * M2]

                # one accumulation group per PSUM bank: start=True clears the
                # whole bank, so only the first (even-d) bias matmul starts it;
                # the odd-d region overwrites via clear has_written bits
                for d in range(NB):
                    nc.tensor.matmul(
                        out=at_slice(d),
                        lhsT=bout_sb[0:1, d * P:(d + 1) * P],
                        rhs=ones_sb[0:1, :M2],
                        start=(d % 2 == 0), stop=False,
                    )
                # clT blocks first: they are ready before the softmax/ctx
                # chain resolves, keeping the PE warm through that window
                f_order = list(range(NB, 2 * NB)) + list(range(NB))
                w_row = None
                for fi, f in enumerate(f_order):
                    if fi % 2 == 0:
                        fa, fb = f_order[fi], f_order[fi + 1]
                        w_row = wrp.tile([P, 2 * H], wdt, tag="wout", name="wout")
                        nc.sync.dma_start(
                            out=w_row[:, :].rearrange("p (b c) -> p b c", b=2),
                            in_=Wout[fa * P:(fa + 2) * P, :].rearrange(
                                "(b p) c -> p b c", p=P
                            ),
                        )
                    rhs_blk = ctxT[f] if f < NB else clT[f - NB]
                    off = (fi % 2) * H
                    for d in range(NB):
                        nc.tensor.matmul(
                            out=at_slice(d),
                            lhsT=w_row[:, off + d * P:off + (d + 1) * P],
                            rhs=rhs_blk[:, :],
                            start=False,
                            stop=(fi == 2 * NB - 1 and d % 2 == 1),
                        )
                for d in range(NB):
                    nc.vector.tensor_copy(out=attnT[d][:, :], in_=at_slice(d))

            # ---- mrg: mrg[m, d] = tanh(sum_f cat2T[f, m] * W_mrg[f, d] + b_mrg[d])
            # cat2T blocks: f 0..NB-1 -> clT, NB..2NB-1 -> attnT
            mrg_sb = [pp.tile([MT, H], F32, tag=f"mrg{e}", name=f"mrg{e}") for e in range(BL)]
            ND2 = H // 512 if H >= 512 else 1
            DW = min(H, 512)
            with tc.tile_pool(name="wrow2", bufs=3) as wr2, \
                 tc.tile_pool(name="mg_psum", bufs=1, space="PSUM") as mgp:
                mg_p = [mgp.tile([MT, H], F32, tag=f"mg{e}", name=f"mg{e}") for e in range(BL)]
                for e in range(BL):
                    for d2 in range(ND2):
                        nc.tensor.matmul(
                            out=mg_p[e][:, d2 * DW:(d2 + 1) * DW],
                            lhsT=ones_sb[0:1, :MT],
                            rhs=bmrg_sb[0:1, d2 * DW:(d2 + 1) * DW],
                            start=True, stop=False,
                        )
                w_row = None
                for f in range(2 * NB):
                    lhs_blk = clT[f] if f < NB else attnT[f - NB]
                    if f % 2 == 0:
                        w_row = wr2.tile([P, 2 * H], wdt, tag="wmrg", name="wmrg")
                        nc.sync.dma_start(
                            out=w_row[:, :].rearrange("p (b c) -> p b c", b=2),
                            in_=Wmrg[f * P:(f + 2) * P, :].rearrange(
                                "(b p) c -> p b c", p=P
                            ),
                        )
                    off = (f % 2) * H
                    for e in range(BL):
                        for d2 in range(ND2):
                            nc.tensor.matmul(
                                out=mg_p[e][:, d2 * DW:(d2 + 1) * DW],
                                lhsT=lhs_blk[:, e * MT:(e + 1) * MT],
                                rhs=w_row[:, off + d2 * DW:off + (d2 + 1) * DW],
                                start=False, stop=(f == 2 * NB - 1),
                            )
                for e in range(BL):
                    nc.scalar.activation(
                        out=mrg_sb[e][:, :], in_=mg_p[e][:, :], func=AF.Tanh
                    )

            # ---- scatter merged rows into out (after passthrough copy: WAW)
            for e in range(BL):
                nc.gpsimd.indirect_dma_start(
                    out=out2d,
                    out_offset=IndirectOffsetOnAxis(ap=idx_sb[:, e:e + 1], axis=0),
                    in_=mrg_sb[e][:, :],
                    in_offset=None,
                )

    return nc


# ---------------------------------------------------------------------------

S, B, H, K, C = 1024, 16, 1024, 8, 16
N_CORES = 8
BL = B // N_CORES
WEIGHT_DTYPE = mybir.dt.bfloat16  # F32 for exact; bf16 halves weight HBM + 2x PE

_prog_cache = {}


def _np_wdt():
    return mybir.dt.np(WEIGHT_DTYPE)


def _get_program():
    key = (S, BL, H, K, C, WEIGHT_DTYPE)
    if key not in _prog_cache:
        nc = build_program(S, BL, H, K, C, wdt=WEIGHT_DTYPE)
        nc.finalize()  # Bacc.finalize: wait-splitting, reg alloc, codegen
        _prog_cache[key] = nc
    return _prog_cache[key]


def make_in_maps(m_bank, coref_posi, Wq, Uk, b_attn, v_attn, W_out, b_out,
                 W_mrg, b_mrg):
    MT = K * C
    m_bank = np.ascontiguousarray(m_bank, dtype=np.float32)
    in_maps = []
    for c in range(N_CORES):
        mb_c = np.ascontiguousarray(m_bank[:, c * BL:(c + 1) * BL, :])
        # idx[m, e]: row of mention m of local example e in the (S*BL, H) view
        idx_c = np.empty((MT, BL), dtype=np.int32)
        for e in range(BL):
            pos = np.asarray(coref_posi[c * BL + e], dtype=np.int64).reshape(MT)
            idx_c[:, e] = (pos * BL + e).astype(np.int32)
        in_maps.append({
            "mb": mb_c,
            "idx": idx_c,
            "Wq": np.ascontiguousarray(Wq, dtype=_np_wdt()),
            "Uk": np.ascontiguousarray(Uk, dtype=_np_wdt()),
            "vattn": np.ascontiguousarray(v_attn, dtype=_np_wdt()).reshape(H),
            "battn": np.ascontiguousarray(b_attn, dtype=_np_wdt()).reshape(1, H),
            "Wout": np.ascontiguousarray(W_out, dtype=_np_wdt()),
            "bout": np.ascontiguousarray(b_out, dtype=_np_wdt()).reshape(1, H),
            "Wmrg": np.ascontiguousarray(W_mrg, dtype=_np_wdt()),
            "bmrg": np.ascontiguousarray(b_mrg, dtype=_np_wdt()).reshape(1, H),
        })
    return in_maps


def run(in_maps, trace=False, tmpdir=None):
    from concourse.bass_utils import run_bass_kernel_spmd
    nc = _get_program()
    return run_bass_kernel_spmd(
        nc, in_maps, list(range(N_CORES)), trace=trace, tmpdir=tmpdir
    )


def kernel(**inputs):
    in_maps = make_in_maps(**inputs)
    res = run(in_maps)
    outs = [res.results[c]["out"] for c in range(N_CORES)]
    return np.concatenate(outs, axis=1).astype(np.float32)


if __name__ == "__main__":
    nc = build_program()
    print("program built ok; instructions:",
          sum(len(bb.instructions) for f in nc.m.functions for bb in f.basicblocks)
          if hasattr(nc.m.functions[0], "basicblocks") else "n/a")


# revision 21
# speedup vs baseline: 1.0825x; 1.0825x over previous
"""Trainium2 Bass kernel for nn_CorefMergeLayer.

Reference semantics (per example b):
    cl = m_bank[coref_posi[b], b, :]            # [K, C, H] gathered mentions
    q = cl @ Wq ; k = cl @ Uk
    scores[k,i,j] = v . tanh(q_i + k_j + b_attn)
    alpha = softmax_j(scores)
    ctx = alpha @ cl
    attn_h = [ctx; cl] @ W_out + b_out
    mrg = tanh([cl; attn_h] @ W_mrg + b_mrg)
    out = m_bank with mention rows replaced by mrg

Sharding: data-parallel over batch B=16 across 8 cores (BL=2 examples per
core); weights replicated; W_out/W_mrg/Wq/Uk streamed from HBM.
"""

import sys

for _p in ("/opt/trn_rl_repo",):
    if _p not in sys.path:
        sys.path.insert(0, _p)

import numpy as np

import concourse.bacc as bacc
import concourse.bass as bass
import concourse.mybir as mybir
import concourse.tile as tile
from concourse.bass import IndirectOffsetOnAxis
from concourse.masks import make_identity
import bass_rust as _bass_rust


def _vec_pairs(dims):
    return _bass_rust.VecI64Pair([list(d) for d in dims])


F32 = mybir.dt.float32
I32 = mybir.dt.int32
AF = mybir.ActivationFunctionType
ALU = mybir.AluOpType
AX = mybir.AxisListType

P = 128  # partitions


def build_program(S=1024, BL=2, H=1024, K=8, C=16, wdt=F32):
    """Build the SPMD per-core Bass program.

    Per-core inputs:
      mb    [S, BL, H] f32   batch slice of m_bank
      idx   [MT, BL]   i32   row indices into the (S*BL, H) view of mb
      Wq,Uk [H, H]     f32
      vattn [H]        f32
      battn,bout,bmrg [1, H] f32
      W_out,W_mrg [2H, H] f32
    Output:
      out   [S, BL, H] f32
    """
    MT = K * C                 # mentions per example (<= 128)
    M2 = BL * MT               # mention columns, both examples
    NB = H // P                # h blocks
    PAIRS_E = K * C * C        # pair columns per example
    assert MT <= P and H % P == 0 and M2 <= 512

    nc = bacc.Bacc()

    mb = nc.dram_tensor("mb", [S, BL, H], F32, kind="ExternalInput")
    idx = nc.dram_tensor("idx", [MT, BL], I32, kind="ExternalInput")
    Wq = nc.dram_tensor("Wq", [H, H], wdt, kind="ExternalInput")
    Uk = nc.dram_tensor("Uk", [H, H], wdt, kind="ExternalInput")
    vattn = nc.dram_tensor("vattn", [H], wdt, kind="ExternalInput")
    battn = nc.dram_tensor("battn", [1, H], wdt, kind="ExternalInput")
    Wout = nc.dram_tensor("Wout", [2 * H, H], wdt, kind="ExternalInput")
    bout = nc.dram_tensor("bout", [1, H], wdt, kind="ExternalInput")
    Wmrg = nc.dram_tensor("Wmrg", [2 * H, H], wdt, kind="ExternalInput")
    bmrg = nc.dram_tensor("bmrg", [1, H], wdt, kind="ExternalInput")
    out = nc.dram_tensor("out", [S, BL, H], F32, kind="ExternalOutput")

    mb2d = mb[:, :, :].rearrange("s b h -> (s b) h")
    out2d = out[:, :, :].rearrange("s b h -> (s b) h")

    with tile.TileContext(nc) as tc:
        with tc.tile_pool(name="persist", bufs=1) as pp:
            # ---- passthrough copy mb -> out (DRAM->DRAM), issued first so
            # it overlaps all compute; the final scatter depends on it (WAW).
            ncopy = 2
            rows = S // ncopy
            for i in range(ncopy):
                nc.scalar.dma_start(
                    out=out[i * rows:(i + 1) * rows, :, :],
                    in_=mb[i * rows:(i + 1) * rows, :, :],
                )

            # ---- constants into SBUF
            idx_sb = pp.tile([MT, BL], I32, tag="idx", name="idx_sb")
            nc.sync.dma_start(out=idx_sb[:, :], in_=idx[:, :])

            vT_sb = pp.tile([P, NB], wdt, tag="vT", name="vT_sb")  # vT[p, nb] = v[nb*128+p]
            nc.sync.dma_start(
                out=vT_sb[:, :], in_=vattn[:].rearrange("(nb p) -> p nb", p=P)
            )

            battn_sb = pp.tile([1, H], wdt, tag="battn", name="battn_sb")
            nc.sync.dma_start(out=battn_sb[:, :], in_=battn[:, :])
            bout_sb = pp.tile([1, H], wdt, tag="bout", name="bout_sb")
            nc.sync.dma_start(out=bout_sb[:, :], in_=bout[:, :])
            bmrg_sb = pp.tile([1, H], wdt, tag="bmrg", name="bmrg_sb")
            nc.sync.dma_start(out=bmrg_sb[:, :], in_=bmrg[:, :])

            # ---- gather mentions: cl[e] [MT, H] mention-major.
            # These go FIRST on the gpsimd queue: the whole PE pipeline
            # (transpose -> projections) waits on them.
            cl_sb = [pp.tile([MT, H], F32, tag=f"cl{e}", name=f"cl{e}") for e in range(BL)]
            for e in range(BL):
                nc.gpsimd.indirect_dma_start(
                    out=cl_sb[e][:, :],
                    out_offset=None,
                    in_=mb2d,
                    in_offset=IndirectOffsetOnAxis(ap=idx_sb[:, e:e + 1], axis=0),
                )

            ident = pp.tile([P, P], F32, tag="ident", name="ident")
            make_identity(nc, ident[:, :])

            # bf16 copies of cl for the ctx matmul (lhsT dtype must match abd)
            cl_bf = [pp.tile([MT, H], wdt, tag=f"clbf{e}", name=f"clbf{e}")
                     for e in range(BL)]
            for e in range(BL):
                nc.vector.tensor_copy(out=cl_bf[e][:, :], in_=cl_sb[e][:, :])

            ones_sb = pp.tile([1, max(M2, P)], wdt, tag="ones", name="ones_sb")
            nc.gpsimd.memset(ones_sb[:, :], 1.0)

            # row images of the block-diagonal alphaT matrices (partition 0);
            # zeroed once early -- only the diagonal blocks are ever
            # rewritten.  One per example so the two softmaxes overlap.
            diag = [pp.tile([1, MT * MT], wdt, tag=f"diag{e}", name=f"diag{e}")
                    for e in range(BL)]
            for e in range(BL):
                nc.gpsimd.memset(diag[e][:, :], 0.0)

            # ---- transpose to clT blocks [128, M2]
            clT = [pp.tile([P, M2], wdt, tag=f"clT{b}", name=f"clT{b}") for b in range(NB)]
            with tc.tile_pool(name="tp_psum", bufs=3, space="PSUM") as tpp:
                for e in range(BL):
                    for b in range(NB):
                        tp = tpp.tile([P, MT], F32, tag="tp", name="tp")
                        nc.tensor.transpose(
                            out=tp[:, :],
                            in_=cl_sb[e][:, b * P:(b + 1) * P],
                            identity=ident[:MT, :MT],
                        )
                        nc.vector.tensor_copy(
                            out=clT[b][:, e * MT:(e + 1) * MT], in_=tp[:, :]
                        )

            # ---- projections: QT/KT blocks [128, M2];  KT += b_attn
            QT = [pp.tile([P, M2], wdt, tag=f"QT{b}", name=f"QT{b}") for b in range(NB)]
            KT = [pp.tile([P, M2], wdt, tag=f"KT{b}", name=f"KT{b}") for b in range(NB)]
            with tc.tile_pool(name="wcol", bufs=2) as wp, \
                 tc.tile_pool(name="qk_psum", bufs=2, space="PSUM") as qkp:
                # paired column loads: one DMA covers two ho blocks
                # wq_col[p, hi*2P + c] = Wq[hi*128 + p, ho2*2P + c], c in [0, 2P)
                wq_col = uk_col = None
                for ho in range(NB):
                    if ho % 2 == 0:
                        ho2 = ho // 2
                        wq_col = wp.tile([P, 2 * H], wdt, tag="wq", name="wq")
                        nc.scalar.dma_start(
                            out=wq_col[:, :].rearrange("p (hi c) -> p hi c", hi=NB),
                            in_=Wq[:, ho2 * 2 * P:(ho2 + 1) * 2 * P].rearrange(
                                "(hi p) c -> p hi c", p=P
                            ),
                        )
                        uk_col = wp.tile([P, 2 * H], wdt, tag="uk", name="uk")
                        nc.scalar.dma_start(
                            out=uk_col[:, :].rearrange("p (hi c) -> p hi c", hi=NB),
                            in_=Uk[:, ho2 * 2 * P:(ho2 + 1) * 2 * P].rearrange(
                                "(hi p) c -> p hi c", p=P
                            ),
                        )
                    off = (ho % 2) * P
                    qt_p = qkp.tile([P, M2], F32, tag="qt", name="qt")
                    kt_p = qkp.tile([P, M2], F32, tag="kt", name="kt")
                    for hi in range(NB):
                        nc.tensor.matmul(
                            out=qt_p[:, :],
                            lhsT=wq_col[:, hi * 2 * P + off:hi * 2 * P + off + P],
                            rhs=clT[hi][:, :],
                            start=(hi == 0), stop=(hi == NB - 1),
                        )
                    # KT = b_attn (rank-1: b_chunk^T @ ones), then += Uk.T cl
                    nc.tensor.matmul(
                        out=kt_p[:, :],
                        lhsT=battn_sb[0:1, ho * P:(ho + 1) * P],
                        rhs=ones_sb[0:1, :M2],
                        start=True, stop=False,
                    )
                    for hi in range(NB):
                        nc.tensor.matmul(
                            out=kt_p[:, :],
                            lhsT=uk_col[:, hi * 2 * P + off:hi * 2 * P + off + P],
                            rhs=clT[hi][:, :],
                            start=False, stop=(hi == NB - 1),
                        )
                    nc.vector.tensor_copy(out=QT[ho][:, :], in_=qt_p[:, :])
                    nc.scalar.activation(out=KT[ho][:, :], in_=kt_p[:, :], func=AF.Copy)

            # ---- pair scores: sc[e][(k,j,i)] = sum_h v[h]*tanh(q_i+k_j+b)
            NCH = (PAIRS_E + 511) // 512  # 512-col psum chunks per example
            with tc.tile_pool(name="sc_psum", bufs=1, space="PSUM") as scp, \
                 tc.tile_pool(name="epool", bufs=2) as ep:
                sc_p = [scp.tile([1, PAIRS_E], F32, tag=f"sc{e}", name=f"sc{e}") for e in range(BL)]
                for hb in range(NB):
                    for e in range(BL):
                        e_in = ep.tile([P, PAIRS_E], wdt, tag="e_in", name="e_in")
                        kin = KT[hb][:, e * MT:(e + 1) * MT] \
                            .rearrange("p (k j) -> p k j", k=K) \
                            .unsqueeze(3).to_broadcast([P, K, C, C])
                        qin = QT[hb][:, e * MT:(e + 1) * MT] \
                            .rearrange("p (k i) -> p k i", k=K) \
                            .unsqueeze(2).to_broadcast([P, K, C, C])
                        eng = nc.vector if e == 0 else nc.gpsimd
                        eng.tensor_tensor(
                            out=e_in[:, :].rearrange("p (k j i) -> p k j i", k=K, j=C),
                            in0=kin, in1=qin, op=ALU.add,
                        )
                        e_t = ep.tile([P, PAIRS_E], wdt, tag="e_t", name="e_t", bufs=1)
                        nc.scalar.activation(out=e_t[:, :], in_=e_in[:, :], func=AF.Tanh)
                        for ch in range(NCH):
                            c0, c1 = ch * 512, min((ch + 1) * 512, PAIRS_E)
                            nc.tensor.matmul(
                                out=sc_p[e][0:1, c0:c1],
                                lhsT=vT_sb[:, hb:hb + 1],
                                rhs=e_t[:, c0:c1],
                                start=(hb == 0), stop=(hb == NB - 1),
                            )

                # ---- softmax over j; alpha laid out (k, j, i)
                abd = [pp.tile([MT, MT], wdt, tag=f"abd{e}", name=f"abd{e}") for e in range(BL)]
                for e in range(BL):
                    # exp written strided into the pre-zeroed row image of the
                    # block-diagonal alphaT matrix: position of value (k,j,i)
                    # is (k*C+j)*MT + k*C + i (affine in (k,j,i)).  After the
                    # in-place normalize, one DMA reshapes the row image to
                    # the [MT, MT] tile, so the ctx matmul depends on a
                    # single DMA (HW sync-wait limit).
                    p0 = list(list(diag[e][0:1, :].ap)[0])
                    dg_kji = diag[e][0:1, :].copy()
                    dg_kji.ap = _vec_pairs([p0, [C * MT + C, K], [MT, C], [1, C]])
                    dg_kij = diag[e][0:1, :].copy()
                    dg_kij.ap = _vec_pairs([p0, [C * MT + C, K], [1, C], [MT, C]])
                    nc.scalar.activation(
                        out=dg_kji, in_=sc_p[e][0:1, :].rearrange(
                            "p (k j i) -> p k j i", k=K, j=C),
                        func=AF.Exp,
                    )
                    sum_sb = pp.tile([1, K * C], F32, tag=f"sum{e}", name=f"sumsb{e}")
                    nc.vector.tensor_reduce(
                        out=sum_sb[:, :], in_=dg_kij, axis=AX.X, op=ALU.add,
                    )
                    rs_sb = pp.tile([1, K * C], F32, tag=f"rs{e}", name=f"rssb{e}")
                    nc.vector.reciprocal(out=rs_sb[:, :], in_=sum_sb[:, :])
                    nc.vector.tensor_tensor(
                        out=dg_kji, in0=dg_kji,
                        in1=rs_sb[:, :].rearrange("p (k i) -> p k i", k=K)
                            .unsqueeze(2).to_broadcast([1, K, C, C]),
                        op=ALU.mult,
                    )
                    nc.sync.dma_start(out=abd[e][:, :], in_=diag[e][0:1, :])

            # ---- ctx: ctxT[h, (k,i)] = sum_(k,j) cl[(k,j), h] * abd[(k,j),(k,i)]
            ctxT = [pp.tile([P, M2], wdt, tag=f"ctxT{b}", name=f"ctxT{b}") for b in range(NB)]
            with tc.tile_pool(name="ctx_psum", bufs=3, space="PSUM") as cxp:
                for e in range(BL):
                    for b in range(NB):
                        cx = cxp.tile([P, MT], F32, tag="cx", name="cx")
                        nc.tensor.matmul(
                            out=cx[:, :],
                            lhsT=cl_bf[e][:, b * P:(b + 1) * P],
                            rhs=abd[e][:, :],
                            start=True, stop=True,
                        )
                        nc.vector.tensor_copy(
                            out=ctxT[b][:, e * MT:(e + 1) * MT], in_=cx[:, :]
                        )

            # ---- attn_h: attnT[d, m] = sum_f W_out[f, d] * cat1T[f, m] + b_out[d]
            # cat1T blocks: f 0..NB-1 -> ctxT, NB..2NB-1 -> clT
            attnT = [pp.tile([P, M2], wdt, tag=f"attnT{b}", name=f"attnT{b}") for b in range(NB)]
            with tc.tile_pool(name="wrow", bufs=3) as wrp, \
                 tc.tile_pool(name="at_psum", bufs=1, space="PSUM") as atp:
                # two d-blocks per PSUM bank tile -> 4 banks, leaving room for
                # the mrg-phase PSUM so its clT half overlaps this phase
                at_p = [atp.tile([P, 2 * M2], F32, tag=f"at{d}", name=f"at{d}")
                        for d in range(NB // 2)]

                def at_slice(d):
                    return at_p[d // 2][:, (d % 2) * M2:(d % 2 + 1) * M2]

                # one accumulation group per PSUM bank: start=True clears the
                # whole bank, so only the first (even-d) bias matmul starts it;
                # the odd-d region overwrites via clear has_written bits
                for d in range(NB):
                    nc.tensor.matmul(
                        out=at_slice(d),
                        lhsT=bout_sb[0:1, d * P:(d + 1) * P],
                        rhs=ones_sb[0:1, :M2],
                        start=(d % 2 == 0), stop=False,
                    )
                # clT blocks first: they are ready before the softmax/ctx
                # chain resolves, keeping the PE warm through that window
                f_order = list(range(NB, 2 * NB)) + list(range(NB))
                w_row = None
                for fi, f in enumerate(f_order):
                    if fi % 2 == 0:
                        fa, fb = f_order[fi], f_order[fi + 1]
                        w_row = wrp.tile([P, 2 * H], wdt, tag="wout", name="wout")
                        nc.sync.dma_start(
                            out=w_row[:, :].rearrange("p (b c) -> p b c", b=2),
                            in_=Wout[fa * P:(fa + 2) * P, :].rearrange(
                                "(b p) c -> p b c", p=P
                            ),
                        )
                    rhs_blk = ctxT[f] if f < NB else clT[f - NB]
                    off = (fi % 2) * H
                    for d in range(NB):
                        nc.tensor.matmul(
                            out=at_slice(d),
                            lhsT=w_row[:, off + d * P:off + (d + 1) * P],
                            rhs=rhs_blk[:, :],
                            start=False,
                            stop=(fi == 2 * NB - 1 and d % 2 == 1),
                        )
                for d in range(NB):
                    nc.vector.tensor_copy(out=attnT[d][:, :], in_=at_slice(d))

            # ---- mrg: mrg[m, d] = tanh(sum_f cat2T[f, m] * W_mrg[f, d] + b_mrg[d])
            # cat2T blocks: f 0..NB-1 -> clT, NB..2NB-1 -> attnT
            mrg_sb = [pp.tile([MT, H], F32, tag=f"mrg{e}", name=f"mrg{e}") for e in range(BL)]
            ND2 = H // 512 if H >= 512 else 1
            DW = min(H, 512)
            with tc.tile_pool(name="wrow2", bufs=3) as wr2, \
                 tc.tile_pool(name="mg_psum", bufs=1, space="PSUM") as mgp:
                mg_p = [mgp.tile([MT, H], F32, tag=f"mg{e}", name=f"mg{e}") for e in range(BL)]
                for e in range(BL):
                    for d2 in range(ND2):
                        nc.tensor.matmul(
                            out=mg_p[e][:, d2 * DW:(d2 + 1) * DW],
                            lhsT=ones_sb[0:1, :MT],
                            rhs=bmrg_sb[0:1, d2 * DW:(d2 + 1) * DW],
                            start=True, stop=False,
                        )
                w_row = None
                for f in range(2 * NB):
                    lhs_blk = clT[f] if f < NB else attnT[f - NB]
                    if f % 2 == 0:
                        w_row = wr2.tile([P, 2 * H], wdt, tag="wmrg", name="wmrg")
                        nc.sync.dma_start(
                            out=w_row[:, :].rearrange("p (b c) -> p b c", b=2),
                            in_=Wmrg[f * P:(f + 2) * P, :].rearrange(
                                "(b p) c -> p b c", p=P
                            ),
                        )
                    off = (f % 2) * H
                    for e in range(BL):
                        for d2 in range(ND2):
                            nc.tensor.matmul(
                                out=mg_p[e][:, d2 * DW:(d2 + 1) * DW],
                                lhsT=lhs_blk[:, e * MT:(e + 1) * MT],
                                rhs=w_row[:, off + d2 * DW:off + (d2 + 1) * DW],
                                start=False, stop=(f == 2 * NB - 1),
                            )
                for e in range(BL):
                    nc.scalar.activation(
                        out=mrg_sb[e][:, :], in_=mg_p[e][:, :], func=AF.Tanh
                    )

            # ---- scatter merged rows into out (after passthrough copy: WAW)
            for e in range(BL):
                nc.gpsimd.indirect_dma_start(
                    out=out2d,
                    out_offset=IndirectOffsetOnAxis(ap=idx_sb[:, e:e + 1], axis=0),
                    in_=mrg_sb[e][:, :],
                    in_offset=None,
                )

    return nc


# ---------------------------------------------------------------------------

S, B, H, K, C = 1024, 16, 1024, 8, 16
N_CORES = 8
BL = B // N_CORES
WEIGHT_DTYPE = mybir.dt.bfloat16  # F32 for exact; bf16 halves weight HBM + 2x PE

_prog_cache = {}


def _np_wdt():
    return mybir.dt.np(WEIGHT_DTYPE)


def _get_program():
    key = (S, BL, H, K, C, WEIGHT_DTYPE)
    if key not in _prog_cache:
        nc = build_program(S, BL, H, K, C, wdt=WEIGHT_DTYPE)
        nc.finalize()  # Bacc.finalize: wait-splitting, reg alloc, codegen
        _prog_cache[key] = nc
    return _prog_cache[key]


def make_in_maps(m_bank, coref_posi, Wq, Uk, b_attn, v_attn, W_out, b_out,
                 W_mrg, b_mrg):
    MT = K * C
    m_bank = np.ascontiguousarray(m_bank, dtype=np.float32)
    in_maps = []
    for c in range(N_CORES):
        mb_c = np.ascontiguousarray(m_bank[:, c * BL:(c + 1) * BL, :])
        # idx[m, e]: row of mention m of local example e in the (S*BL, H) view
        idx_c = np.empty((MT, BL), dtype=np.int32)
        for e in range(BL):
            pos = np.asarray(coref_posi[c * BL + e], dtype=np.int64).reshape(MT)
            idx_c[:, e] = (pos * BL + e).astype(np.int32)
        in_maps.append({
            "mb": mb_c,
            "idx": idx_c,
            "Wq": np.ascontiguousarray(Wq, dtype=_np_wdt()),
            "Uk": np.ascontiguousarray(Uk, dtype=_np_wdt()),
            "vattn": np.ascontiguousarray(v_attn, dtype=_np_wdt()).reshape(H),
            "battn": np.ascontiguousarray(b_attn, dtype=_np_wdt()).reshape(1, H),
            "Wout": np.ascontiguousarray(W_out, dtype=_np_wdt()),
            "bout": np.ascontiguousarray(b_out, dtype=_np_wdt()).reshape(1, H),
            "Wmrg": np.ascontiguousarray(W_mrg, dtype=_np_wdt()),
            "bmrg": np.ascontiguousarray(b_mrg, dtype=_np_wdt()).reshape(1, H),
        })
    return in_maps


def run(in_maps, trace=False, tmpdir=None):
    from concourse.bass_utils import run_bass_kernel_spmd
    nc = _get_program()
    return run_bass_kernel_spmd(
        nc, in_maps, list(range(N_CORES)), trace=trace, tmpdir=tmpdir
    )


def kernel(**inputs):
    in_maps = make_in_maps(**inputs)
    res = run(in_maps)
    outs = [res.results[c]["out"] for c in range(N_CORES)]
    return np.concatenate(outs, axis=1).astype(np.float32)


if __name__ == "__main__":
    nc = build_program()
    print("program built ok; instructions:",
          sum(len(bb.instructions) for f in nc.m.functions for bb in f.basicblocks)
          if hasattr(nc.m.functions[0], "basicblocks") else "n/a")


# revision 22
# speedup vs baseline: 1.1833x; 1.0931x over previous
"""Trainium2 Bass kernel for nn_CorefMergeLayer.

Reference semantics (per example b):
    cl = m_bank[coref_posi[b], b, :]            # [K, C, H] gathered mentions
    q = cl @ Wq ; k = cl @ Uk
    scores[k,i,j] = v . tanh(q_i + k_j + b_attn)
    alpha = softmax_j(scores)
    ctx = alpha @ cl
    attn_h = [ctx; cl] @ W_out + b_out
    mrg = tanh([cl; attn_h] @ W_mrg + b_mrg)
    out = m_bank with mention rows replaced by mrg

Sharding: data-parallel over batch B=16 across 8 cores (BL=2 examples per
core); weights replicated; W_out/W_mrg/Wq/Uk streamed from HBM.
"""

import sys

for _p in ("/opt/trn_rl_repo",):
    if _p not in sys.path:
        sys.path.insert(0, _p)

import numpy as np

import concourse.bacc as bacc
import concourse.bass as bass
import concourse.mybir as mybir
import concourse.tile as tile
from concourse.bass import IndirectOffsetOnAxis
from concourse.masks import make_identity
import bass_rust as _bass_rust


def _vec_pairs(dims):
    return _bass_rust.VecI64Pair([list(d) for d in dims])


F32 = mybir.dt.float32
I32 = mybir.dt.int32
AF = mybir.ActivationFunctionType
ALU = mybir.AluOpType
AX = mybir.AxisListType

P = 128  # partitions


def build_program(S=1024, BL=2, H=1024, K=8, C=16, wdt=F32):
    """Build the SPMD per-core Bass program.

    Per-core inputs:
      mb    [S, BL, H] f32   batch slice of m_bank
      idx   [MT, BL]   i32   row indices into the (S*BL, H) view of mb
      Wq,Uk [H, H]     f32
      vattn [H]        f32
      battn,bout,bmrg [1, H] f32
      W_out,W_mrg [2H, H] f32
    Output:
      out   [S, BL, H] f32
    """
    MT = K * C                 # mentions per example (<= 128)
    M2 = BL * MT               # mention columns, both examples
    NB = H // P                # h blocks
    PAIRS_E = K * C * C        # pair columns per example
    assert MT <= P and H % P == 0 and M2 <= 512

    nc = bacc.Bacc()

    mb = nc.dram_tensor("mb", [S, BL, H], F32, kind="ExternalInput")
    idx = nc.dram_tensor("idx", [MT, BL], I32, kind="ExternalInput")
    Wq = nc.dram_tensor("Wq", [H, H], wdt, kind="ExternalInput")
    Uk = nc.dram_tensor("Uk", [H, H], wdt, kind="ExternalInput")
    vattn = nc.dram_tensor("vattn", [H], wdt, kind="ExternalInput")
    battn = nc.dram_tensor("battn", [1, H], wdt, kind="ExternalInput")
    Wout = nc.dram_tensor("Wout", [2 * H, H], wdt, kind="ExternalInput")
    bout = nc.dram_tensor("bout", [1, H], wdt, kind="ExternalInput")
    Wmrg = nc.dram_tensor("Wmrg", [2 * H, H], wdt, kind="ExternalInput")
    bmrg = nc.dram_tensor("bmrg", [1, H], wdt, kind="ExternalInput")
    out = nc.dram_tensor("out", [S, BL, H], F32, kind="ExternalOutput")

    mb2d = mb[:, :, :].rearrange("s b h -> (s b) h")
    out2d = out[:, :, :].rearrange("s b h -> (s b) h")

    with tile.TileContext(nc) as tc:
        with tc.tile_pool(name="persist", bufs=1) as pp:
            # ---- passthrough copy mb -> out (DRAM->DRAM), issued first so
            # it overlaps all compute; the final scatter depends on it (WAW).
            ncopy = 2
            rows = S // ncopy
            for i in range(ncopy):
                nc.scalar.dma_start(
                    out=out[i * rows:(i + 1) * rows, :, :],
                    in_=mb[i * rows:(i + 1) * rows, :, :],
                )

            # ---- constants into SBUF
            idx_sb = pp.tile([MT, BL], I32, tag="idx", name="idx_sb")
            nc.sync.dma_start(out=idx_sb[:, :], in_=idx[:, :])

            vT_sb = pp.tile([P, NB], wdt, tag="vT", name="vT_sb")  # vT[p, nb] = v[nb*128+p]
            nc.sync.dma_start(
                out=vT_sb[:, :], in_=vattn[:].rearrange("(nb p) -> p nb", p=P)
            )

            battn_sb = pp.tile([1, H], wdt, tag="battn", name="battn_sb")
            nc.sync.dma_start(out=battn_sb[:, :], in_=battn[:, :])
            bout_sb = pp.tile([1, H], wdt, tag="bout", name="bout_sb")
            nc.sync.dma_start(out=bout_sb[:, :], in_=bout[:, :])
            bmrg_sb = pp.tile([1, H], wdt, tag="bmrg", name="bmrg_sb")
            nc.sync.dma_start(out=bmrg_sb[:, :], in_=bmrg[:, :])

            # ---- gather mentions: cl[e] [MT, H] mention-major.
            # These go FIRST on the gpsimd queue: the whole PE pipeline
            # (transpose -> projections) waits on them.
            cl_sb = [pp.tile([MT, H], F32, tag=f"cl{e}", name=f"cl{e}") for e in range(BL)]
            for e in range(BL):
                nc.gpsimd.indirect_dma_start(
                    out=cl_sb[e][:, :],
                    out_offset=None,
                    in_=mb2d,
                    in_offset=IndirectOffsetOnAxis(ap=idx_sb[:, e:e + 1], axis=0),
                )

            ident = pp.tile([P, P], F32, tag="ident", name="ident")
            make_identity(nc, ident[:, :])

            # bf16 copies of cl for the ctx matmul (lhsT dtype must match abd)
            cl_bf = [pp.tile([MT, H], wdt, tag=f"clbf{e}", name=f"clbf{e}")
                     for e in range(BL)]
            for e in range(BL):
                nc.vector.tensor_copy(out=cl_bf[e][:, :], in_=cl_sb[e][:, :])

            ones_sb = pp.tile([1, max(M2, P)], wdt, tag="ones", name="ones_sb")
            nc.gpsimd.memset(ones_sb[:, :], 1.0)

            # row images of the block-diagonal alphaT matrices (partition 0);
            # zeroed once early -- only the diagonal blocks are ever
            # rewritten.  One per example so the two softmaxes overlap.
            # Zeroing a [1, MT*MT] single-partition tile with memset is ~14us
            # (one lane); instead memset a [MT, MT] tile across partitions
            # (fast) and DMA-flatten it into each row image.
            ztile = pp.tile([MT, MT], wdt, tag="ztile", name="ztile")
            nc.gpsimd.memset(ztile[:, :], 0.0)
            diag = [pp.tile([1, MT * MT], wdt, tag=f"diag{e}", name=f"diag{e}")
                    for e in range(BL)]
            for e in range(BL):
                nc.sync.dma_start(out=diag[e][0:1, :], in_=ztile[:, :])

            # ---- transpose to clT blocks [128, M2]
            clT = [pp.tile([P, M2], wdt, tag=f"clT{b}", name=f"clT{b}") for b in range(NB)]
            with tc.tile_pool(name="tp_psum", bufs=3, space="PSUM") as tpp:
                for e in range(BL):
                    for b in range(NB):
                        tp = tpp.tile([P, MT], F32, tag="tp", name="tp")
                        nc.tensor.transpose(
                            out=tp[:, :],
                            in_=cl_sb[e][:, b * P:(b + 1) * P],
                            identity=ident[:MT, :MT],
                        )
                        nc.vector.tensor_copy(
                            out=clT[b][:, e * MT:(e + 1) * MT], in_=tp[:, :]
                        )

            # ---- projections: QT/KT blocks [128, M2];  KT += b_attn
            QT = [pp.tile([P, M2], wdt, tag=f"QT{b}", name=f"QT{b}") for b in range(NB)]
            KT = [pp.tile([P, M2], wdt, tag=f"KT{b}", name=f"KT{b}") for b in range(NB)]
            with tc.tile_pool(name="wcol", bufs=3) as wp, \
                 tc.tile_pool(name="qk_psum", bufs=1, space="PSUM") as qkp:
                # contiguous row-chunk loads: Wq[hi*P:(hi+1)*P, :] IS the lhsT
                # layout for (hi, all ho).  All 8 QT / 8 KT accumulators live
                # in PSUM simultaneously, packed two-per-bank.
                qt_pk = [qkp.tile([P, 2 * M2], F32, tag=f"qtp{i}", name=f"qtp{i}")
                         for i in range(NB // 2)]
                kt_pk = [qkp.tile([P, 2 * M2], F32, tag=f"ktp{i}", name=f"ktp{i}")
                         for i in range(NB // 2)]

                def qt_slice(ho):
                    return qt_pk[ho // 2][:, (ho % 2) * M2:(ho % 2 + 1) * M2]

                def kt_slice(ho):
                    return kt_pk[ho // 2][:, (ho % 2) * M2:(ho % 2 + 1) * M2]

                # KT = b_attn (rank-1) first: one accumulation group per bank
                for ho in range(NB):
                    nc.tensor.matmul(
                        out=kt_slice(ho),
                        lhsT=battn_sb[0:1, ho * P:(ho + 1) * P],
                        rhs=ones_sb[0:1, :M2],
                        start=(ho % 2 == 0), stop=False,
                    )
                for hi in range(NB):
                    wq_row = wp.tile([P, H], wdt, tag="wq", name="wq")
                    nc.scalar.dma_start(
                        out=wq_row[:, :], in_=Wq[hi * P:(hi + 1) * P, :]
                    )
                    uk_row = wp.tile([P, H], wdt, tag="uk", name="uk")
                    nc.scalar.dma_start(
                        out=uk_row[:, :], in_=Uk[hi * P:(hi + 1) * P, :]
                    )
                    for ho in range(NB):
                        nc.tensor.matmul(
                            out=qt_slice(ho),
                            lhsT=wq_row[:, ho * P:(ho + 1) * P],
                            rhs=clT[hi][:, :],
                            start=(hi == 0 and ho % 2 == 0),
                            stop=(hi == NB - 1 and ho % 2 == 1),
                        )
                    for ho in range(NB):
                        nc.tensor.matmul(
                            out=kt_slice(ho),
                            lhsT=uk_row[:, ho * P:(ho + 1) * P],
                            rhs=clT[hi][:, :],
                            start=False,
                            stop=(hi == NB - 1 and ho % 2 == 1),
                        )
                for ho in range(NB):
                    nc.vector.tensor_copy(out=QT[ho][:, :], in_=qt_slice(ho))
                    nc.scalar.activation(out=KT[ho][:, :], in_=kt_slice(ho), func=AF.Copy)

            # ---- pair scores: sc[e][(k,j,i)] = sum_h v[h]*tanh(q_i+k_j+b)
            NCH = (PAIRS_E + 511) // 512  # 512-col psum chunks per example
            with tc.tile_pool(name="sc_psum", bufs=1, space="PSUM") as scp, \
                 tc.tile_pool(name="epool", bufs=2) as ep:
                sc_p = [scp.tile([1, PAIRS_E], F32, tag=f"sc{e}", name=f"sc{e}") for e in range(BL)]
                for hb in range(NB):
                    for e in range(BL):
                        e_in = ep.tile([P, PAIRS_E], wdt, tag="e_in", name="e_in", bufs=4)
                        kin = KT[hb][:, e * MT:(e + 1) * MT] \
                            .rearrange("p (k j) -> p k j", k=K) \
                            .unsqueeze(3).to_broadcast([P, K, C, C])
                        qin = QT[hb][:, e * MT:(e + 1) * MT] \
                            .rearrange("p (k i) -> p k i", k=K) \
                            .unsqueeze(2).to_broadcast([P, K, C, C])
                        eng = nc.vector if e == 0 else nc.gpsimd
                        eng.tensor_tensor(
                            out=e_in[:, :].rearrange("p (k j i) -> p k j i", k=K, j=C),
                            in0=kin, in1=qin, op=ALU.add,
                        )
                        e_t = ep.tile([P, PAIRS_E], wdt, tag="e_t", name="e_t", bufs=4)
                        nc.scalar.activation(out=e_t[:, :], in_=e_in[:, :], func=AF.Tanh)
                        for ch in range(NCH):
                            c0, c1 = ch * 512, min((ch + 1) * 512, PAIRS_E)
                            nc.tensor.matmul(
                                out=sc_p[e][0:1, c0:c1],
                                lhsT=vT_sb[:, hb:hb + 1],
                                rhs=e_t[:, c0:c1],
                                start=(hb == 0), stop=(hb == NB - 1),
                            )

                # ---- softmax over j; alpha laid out (k, j, i)
                abd = [pp.tile([MT, MT], wdt, tag=f"abd{e}", name=f"abd{e}") for e in range(BL)]
                for e in range(BL):
                    # exp written strided into the pre-zeroed row image of the
                    # block-diagonal alphaT matrix: position of value (k,j,i)
                    # is (k*C+j)*MT + k*C + i (affine in (k,j,i)).  After the
                    # in-place normalize, one DMA reshapes the row image to
                    # the [MT, MT] tile, so the ctx matmul depends on a
                    # single DMA (HW sync-wait limit).
                    p0 = list(list(diag[e][0:1, :].ap)[0])
                    dg_kji = diag[e][0:1, :].copy()
                    dg_kji.ap = _vec_pairs([p0, [C * MT + C, K], [MT, C], [1, C]])
                    dg_kij = diag[e][0:1, :].copy()
                    dg_kij.ap = _vec_pairs([p0, [C * MT + C, K], [1, C], [MT, C]])
                    nc.scalar.activation(
                        out=dg_kji, in_=sc_p[e][0:1, :].rearrange(
                            "p (k j i) -> p k j i", k=K, j=C),
                        func=AF.Exp,
                    )
                    sum_sb = pp.tile([1, K * C], F32, tag=f"sum{e}", name=f"sumsb{e}")
                    nc.vector.tensor_reduce(
                        out=sum_sb[:, :], in_=dg_kij, axis=AX.X, op=ALU.add,
                    )
                    rs_sb = pp.tile([1, K * C], F32, tag=f"rs{e}", name=f"rssb{e}")
                    nc.vector.reciprocal(out=rs_sb[:, :], in_=sum_sb[:, :])
                    nc.vector.tensor_tensor(
                        out=dg_kji, in0=dg_kji,
                        in1=rs_sb[:, :].rearrange("p (k i) -> p k i", k=K)
                            .unsqueeze(2).to_broadcast([1, K, C, C]),
                        op=ALU.mult,
                    )
                    nc.sync.dma_start(out=abd[e][:, :], in_=diag[e][0:1, :])

            # ---- ctx: ctxT[h, (k,i)] = sum_(k,j) cl[(k,j), h] * abd[(k,j),(k,i)]
            ctxT = [pp.tile([P, M2], wdt, tag=f"ctxT{b}", name=f"ctxT{b}") for b in range(NB)]
            with tc.tile_pool(name="ctx_psum", bufs=3, space="PSUM") as cxp:
                for e in range(BL):
                    for b in range(NB):
                        cx = cxp.tile([P, MT], F32, tag="cx", name="cx")
                        nc.tensor.matmul(
                            out=cx[:, :],
                            lhsT=cl_bf[e][:, b * P:(b + 1) * P],
                            rhs=abd[e][:, :],
                            start=True, stop=True,
                        )
                        nc.vector.tensor_copy(
                            out=ctxT[b][:, e * MT:(e + 1) * MT], in_=cx[:, :]
                        )

            # ---- attn_h: attnT[d, m] = sum_f W_out[f, d] * cat1T[f, m] + b_out[d]
            # cat1T blocks: f 0..NB-1 -> ctxT, NB..2NB-1 -> clT
            attnT = [pp.tile([P, M2], wdt, tag=f"attnT{b}", name=f"attnT{b}") for b in range(NB)]
            with tc.tile_pool(name="wrow", bufs=3) as wrp, \
                 tc.tile_pool(name="at_psum", bufs=1, space="PSUM") as atp:
                # two d-blocks per PSUM bank tile -> 4 banks, leaving room for
                # the mrg-phase PSUM so its clT half overlaps this phase
                at_p = [atp.tile([P, 2 * M2], F32, tag=f"at{d}", name=f"at{d}")
                        for d in range(NB // 2)]

                def at_slice(d):
                    return at_p[d // 2][:, (d % 2) * M2:(d % 2 + 1) * M2]

                # one accumulation group per PSUM bank: start=True clears the
                # whole bank, so only the first (even-d) bias matmul starts it;
                # the odd-d region overwrites via clear has_written bits
                for d in range(NB):
                    nc.tensor.matmul(
                        out=at_slice(d),
                        lhsT=bout_sb[0:1, d * P:(d + 1) * P],
                        rhs=ones_sb[0:1, :M2],
                        start=(d % 2 == 0), stop=False,
                    )
                # clT blocks first: they are ready before the softmax/ctx
                # chain resolves, keeping the PE warm through that window
                f_order = list(range(NB, 2 * NB)) + list(range(NB))
                w_row = None
                for fi, f in enumerate(f_order):
                    if fi % 2 == 0:
                        fa, fb = f_order[fi], f_order[fi + 1]
                        w_row = wrp.tile([P, 2 * H], wdt, tag="wout", name="wout")
                        nc.sync.dma_start(
                            out=w_row[:, :].rearrange("p (b c) -> p b c", b=2),
                            in_=Wout[fa * P:(fa + 2) * P, :].rearrange(
                                "(b p) c -> p b c", p=P
                            ),
                        )
                    rhs_blk = ctxT[f] if f < NB else clT[f - NB]
                    off = (fi % 2) * H
                    for d in range(NB):
                        nc.tensor.matmul(
                            out=at_slice(d),
                            lhsT=w_row[:, off + d * P:off + (d + 1) * P],
                            rhs=rhs_blk[:, :],
                            start=False,
                            stop=(fi == 2 * NB - 1 and d % 2 == 1),
                        )
                for d in range(NB):
                    nc.vector.tensor_copy(out=attnT[d][:, :], in_=at_slice(d))

            # ---- mrg: mrg[m, d] = tanh(sum_f cat2T[f, m] * W_mrg[f, d] + b_mrg[d])
            # cat2T blocks: f 0..NB-1 -> clT, NB..2NB-1 -> attnT
            mrg_sb = [pp.tile([MT, H], F32, tag=f"mrg{e}", name=f"mrg{e}") for e in range(BL)]
            ND2 = H // 512 if H >= 512 else 1
            DW = min(H, 512)
            with tc.tile_pool(name="wrow2", bufs=3) as wr2, \
                 tc.tile_pool(name="mg_psum", bufs=1, space="PSUM") as mgp:
                mg_p = [mgp.tile([MT, H], F32, tag=f"mg{e}", name=f"mg{e}") for e in range(BL)]
                for e in range(BL):
                    for d2 in range(ND2):
                        nc.tensor.matmul(
                            out=mg_p[e][:, d2 * DW:(d2 + 1) * DW],
                            lhsT=ones_sb[0:1, :MT],
                            rhs=bmrg_sb[0:1, d2 * DW:(d2 + 1) * DW],
                            start=True, stop=False,
                        )
                w_row = None
                for f in range(2 * NB):
                    lhs_blk = clT[f] if f < NB else attnT[f - NB]
                    if f % 2 == 0:
                        w_row = wr2.tile([P, 2 * H], wdt, tag="wmrg", name="wmrg")
                        nc.sync.dma_start(
                            out=w_row[:, :].rearrange("p (b c) -> p b c", b=2),
                            in_=Wmrg[f * P:(f + 2) * P, :].rearrange(
                                "(b p) c -> p b c", p=P
                            ),
                        )
                    off = (f % 2) * H
                    for e in range(BL):
                        for d2 in range(ND2):
                            nc.tensor.matmul(
                                out=mg_p[e][:, d2 * DW:(d2 + 1) * DW],
                                lhsT=lhs_blk[:, e * MT:(e + 1) * MT],
                                rhs=w_row[:, off + d2 * DW:off + (d2 + 1) * DW],
                                start=False, stop=(f == 2 * NB - 1),
                            )
                for e in range(BL):
                    nc.scalar.activation(
                        out=mrg_sb[e][:, :], in_=mg_p[e][:, :], func=AF.Tanh
                    )

            # ---- scatter merged rows into out (after passthrough copy: WAW)
            for e in range(BL):
                nc.gpsimd.indirect_dma_start(
                    out=out2d,
                    out_offset=IndirectOffsetOnAxis(ap=idx_sb[:, e:e + 1], axis=0),
                    in_=mrg_sb[e][:, :],
                    in_offset=None,
                )

    return nc


# ---------------------------------------------------------------------------

S, B, H, K, C = 1024, 16, 1024, 8, 16
N_CORES = 8
BL = B // N_CORES
WEIGHT_DTYPE = mybir.dt.bfloat16  # F32 for exact; bf16 halves weight HBM + 2x PE

_prog_cache = {}


def _np_wdt():
    return mybir.dt.np(WEIGHT_DTYPE)


def _get_program():
    key = (S, BL, H, K, C, WEIGHT_DTYPE)
    if key not in _prog_cache:
        nc = build_program(S, BL, H, K, C, wdt=WEIGHT_DTYPE)
        nc.finalize()  # Bacc.finalize: wait-splitting, reg alloc, codegen
        _prog_cache[key] = nc
    return _prog_cache[key]


def make_in_maps(m_bank, coref_posi, Wq, Uk, b_attn, v_attn, W_out, b_out,
                 W_mrg, b_mrg):
    MT = K * C
    m_bank = np.ascontiguousarray(m_bank, dtype=np.float32)
    in_maps = []
    for c in range(N_CORES):
        mb_c = np.ascontiguousarray(m_bank[:, c * BL:(c + 1) * BL, :])
        # idx[m, e]: row of mention m of local example e in the (S*BL, H) view
        idx_c = np.empty((MT, BL), dtype=np.int32)
        for e in range(BL):
            pos = np.asarray(coref_posi[c * BL + e], dtype=np.int64).reshape(MT)
            idx_c[:, e] = (pos * BL + e).astype(np.int32)
        in_maps.append({
            "mb": mb_c,
            "idx": idx_c,
            "Wq": np.ascontiguousarray(Wq, dtype=_np_wdt()),
            "Uk": np.ascontiguousarray(Uk, dtype=_np_wdt()),
            "vattn": np.ascontiguousarray(v_attn, dtype=_np_wdt()).reshape(H),
            "battn": np.ascontiguousarray(b_attn, dtype=_np_wdt()).reshape(1, H),
            "Wout": np.ascontiguousarray(W_out, dtype=_np_wdt()),
            "bout": np.ascontiguousarray(b_out, dtype=_np_wdt()).reshape(1, H),
            "Wmrg": np.ascontiguousarray(W_mrg, dtype=_np_wdt()),
            "bmrg": np.ascontiguousarray(b_mrg, dtype=_np_wdt()).reshape(1, H),
        })
    return in_maps


def run(in_maps, trace=False, tmpdir=None):
    from concourse.bass_utils import run_bass_kernel_spmd
    nc = _get_program()
    return run_bass_kernel_spmd(
        nc, in_maps, list(range(N_CORES)), trace=trace, tmpdir=tmpdir
    )


def kernel(**inputs):
    in_maps = make_in_maps(**inputs)
    res = run(in_maps)
    outs = [res.results[c]["out"] for c in range(N_CORES)]
    return np.concatenate(outs, axis=1).astype(np.float32)


if __name__ == "__main__":
    nc = build_program()
    print("program built ok; instructions:",
          sum(len(bb.instructions) for f in nc.m.functions for bb in f.basicblocks)
          if hasattr(nc.m.functions[0], "basicblocks") else "n/a")


# revision 29
# speedup vs baseline: 1.2419x; 1.0495x over previous
"""Trainium2 Bass kernel for nn_CorefMergeLayer.

Reference semantics (per example b):
    cl = m_bank[coref_posi[b], b, :]            # [K, C, H] gathered mentions
    q = cl @ Wq ; k = cl @ Uk
    scores[k,i,j] = v . tanh(q_i + k_j + b_attn)
    alpha = softmax_j(scores)
    ctx = alpha @ cl
    attn_h = [ctx; cl] @ W_out + b_out
    mrg = tanh([cl; attn_h] @ W_mrg + b_mrg)
    out = m_bank with mention rows replaced by mrg

Sharding: data-parallel over batch B=16 across 8 cores (BL=2 examples per
core); weights replicated; W_out/W_mrg/Wq/Uk streamed from HBM.
"""

import sys

for _p in ("/opt/trn_rl_repo",):
    if _p not in sys.path:
        sys.path.insert(0, _p)

import numpy as np

import concourse.bacc as bacc
import concourse.bass as bass
import concourse.mybir as mybir
import concourse.tile as tile
from concourse.bass import IndirectOffsetOnAxis
from concourse.masks import make_identity
import bass_rust as _bass_rust


def _vec_pairs(dims):
    return _bass_rust.VecI64Pair([list(d) for d in dims])


F32 = mybir.dt.float32
I32 = mybir.dt.int32
AF = mybir.ActivationFunctionType
ALU = mybir.AluOpType
AX = mybir.AxisListType

P = 128  # partitions


def build_program(S=1024, BL=2, H=1024, K=8, C=16, wdt=F32):
    """Build the SPMD per-core Bass program.

    Per-core inputs:
      mb    [S, BL, H] f32   batch slice of m_bank
      idx   [MT, BL]   i32   row indices into the (S*BL, H) view of mb
      Wq,Uk [H, H]     f32
      vattn [H]        f32
      battn,bout,bmrg [1, H] f32
      W_out,W_mrg [2H, H] f32
    Output:
      out   [S, BL, H] f32
    """
    MT = K * C                 # mentions per example (<= 128)
    M2 = BL * MT               # mention columns, both examples
    NB = H // P                # h blocks
    PAIRS_E = K * C * C        # pair columns per example
    assert MT <= P and H % P == 0 and M2 <= 512

    nc = bacc.Bacc()

    mb = nc.dram_tensor("mb", [S, BL, H], F32, kind="ExternalInput")
    idx = nc.dram_tensor("idx", [MT, BL], I32, kind="ExternalInput")
    Wq = nc.dram_tensor("Wq", [H, H], wdt, kind="ExternalInput")
    Uk = nc.dram_tensor("Uk", [H, H], wdt, kind="ExternalInput")
    vattn = nc.dram_tensor("vattn", [H], wdt, kind="ExternalInput")
    battn = nc.dram_tensor("battn", [1, H], wdt, kind="ExternalInput")
    Wout = nc.dram_tensor("Wout", [2 * H, H], wdt, kind="ExternalInput")
    bout = nc.dram_tensor("bout", [1, H], wdt, kind="ExternalInput")
    Wmrg = nc.dram_tensor("Wmrg", [2 * H, H], wdt, kind="ExternalInput")
    bmrg = nc.dram_tensor("bmrg", [1, H], wdt, kind="ExternalInput")
    out = nc.dram_tensor("out", [S, BL, H], F32, kind="ExternalOutput")

    mb2d = mb[:, :, :].rearrange("s b h -> (s b) h")
    out2d = out[:, :, :].rearrange("s b h -> (s b) h")

    with tile.TileContext(nc) as tc:
        with tc.tile_pool(name="persist", bufs=1) as pp:
            # ---- constants into SBUF
            idx_sb = pp.tile([MT, BL], I32, tag="idx", name="idx_sb")
            nc.sync.dma_start(out=idx_sb[:, :], in_=idx[:, :])

            vT_sb = pp.tile([P, NB], wdt, tag="vT", name="vT_sb")  # vT[p, nb] = v[nb*128+p]

            battn_sb = pp.tile([1, H], wdt, tag="battn", name="battn_sb")
            nc.sync.dma_start(out=battn_sb[:, :], in_=battn[:, :])
            bout_sb = pp.tile([1, H], wdt, tag="bout", name="bout_sb")
            nc.sync.dma_start(out=bout_sb[:, :], in_=bout[:, :])
            bmrg_sb = pp.tile([1, H], wdt, tag="bmrg", name="bmrg_sb")
            nc.sync.dma_start(out=bmrg_sb[:, :], in_=bmrg[:, :])

            # ---- gather mentions: cl[e] [MT, H] mention-major.
            # These go FIRST on the gpsimd queue: the whole PE pipeline
            # (transpose -> projections) waits on them.
            cl_sb = [pp.tile([MT, H], F32, tag=f"cl{e}", name=f"cl{e}") for e in range(BL)]
            for e in range(BL):
                nc.gpsimd.indirect_dma_start(
                    out=cl_sb[e][:, :],
                    out_offset=None,
                    in_=mb2d,
                    in_offset=IndirectOffsetOnAxis(ap=idx_sb[:, e:e + 1], axis=0),
                )

            ident = pp.tile([P, P], F32, tag="ident", name="ident")
            make_identity(nc, ident[:, :])

            # small reshaping loads on the SW queue (strided patterns stall
            # the HWDGE sync queue for ~10us each; SWDGE absorbs them)
            nc.gpsimd.dma_start(
                out=vT_sb[:, :], in_=vattn[:].rearrange("(nb p) -> p nb", p=P)
            )

            # bf16 copies of cl for the ctx matmul (lhsT dtype must match abd)
            cl_bf = [pp.tile([MT, H], wdt, tag=f"clbf{e}", name=f"clbf{e}")
                     for e in range(BL)]
            for e in range(BL):
                nc.vector.tensor_copy(out=cl_bf[e][:, :], in_=cl_sb[e][:, :])

            ones_sb = pp.tile([1, max(M2, P)], wdt, tag="ones", name="ones_sb")
            nc.gpsimd.memset(ones_sb[:, :], 1.0)

            # ---- passthrough copy mb -> out (DRAM->DRAM) on the SW queue:
            # strictly AFTER the gathers in SDMA FIFO order (so they get full
            # bandwidth) and before the final scatters (WAW comes free).
            ncopy = 4
            rows = S // ncopy
            for i in range(ncopy):
                nc.gpsimd.dma_start(
                    out=out[i * rows:(i + 1) * rows, :, :],
                    in_=mb[i * rows:(i + 1) * rows, :, :],
                )

            # row images of the block-diagonal alphaT matrices (partition 0);
            # zeroed once early -- only the diagonal blocks are ever
            # rewritten.  One per example so the two softmaxes overlap.
            # Zeroing a [1, MT*MT] single-partition tile with memset is ~14us
            # (one lane); instead memset a [MT, MT] tile across partitions
            # (fast) and DMA-flatten it into each row image.
            ztile = pp.tile([MT, MT], wdt, tag="ztile", name="ztile")
            nc.gpsimd.memset(ztile[:, :], 0.0)
            diag = [pp.tile([1, MT * MT], wdt, tag=f"diag{e}", name=f"diag{e}")
                    for e in range(BL)]
            for e in range(BL):
                nc.gpsimd.dma_start(out=diag[e][0:1, :], in_=ztile[:, :])

            # resident W_out / W_mrg (bf16): loaded once during the scores
            # window so the attn/mrg phases run without DMA pacing.
            wout_sb = [pp.tile([P, 2 * H], wdt, tag=f"wo{i}", name=f"wo{i}")
                       for i in range(NB)]
            wmrg_sb = [pp.tile([P, 2 * H], wdt, tag=f"wm{i}", name=f"wm{i}")
                       for i in range(NB)]
            for i in range(NB):
                nc.sync.dma_start(
                    out=wout_sb[i][:, :].rearrange("p (b c) -> p b c", b=2),
                    in_=Wout[2 * i * P:(2 * i + 2) * P, :].rearrange(
                        "(b p) c -> p b c", p=P
                    ),
                )
                nc.sync.dma_start(
                    out=wmrg_sb[i][:, :].rearrange("p (b c) -> p b c", b=2),
                    in_=Wmrg[2 * i * P:(2 * i + 2) * P, :].rearrange(
                        "(b p) c -> p b c", p=P
                    ),
                )

            def wout_blk(f):
                return wout_sb[f // 2][:, (f % 2) * H:(f % 2) * H + H]

            def wmrg_blk(f):
                return wmrg_sb[f // 2][:, (f % 2) * H:(f % 2) * H + H]

            # ---- transpose to clT blocks [128, M2]
            clT = [pp.tile([P, M2], wdt, tag=f"clT{b}", name=f"clT{b}") for b in range(NB)]
            with tc.tile_pool(name="tp_psum", bufs=3, space="PSUM") as tpp:
                for e in range(BL):
                    for b in range(NB):
                        tp = tpp.tile([P, MT], F32, tag="tp", name="tp")
                        nc.tensor.transpose(
                            out=tp[:, :],
                            in_=cl_sb[e][:, b * P:(b + 1) * P],
                            identity=ident[:MT, :MT],
                        )
                        nc.vector.tensor_copy(
                            out=clT[b][:, e * MT:(e + 1) * MT], in_=tp[:, :]
                        )

            # ---- projections: QT/KT blocks [128, M2];  KT += b_attn
            QT = [pp.tile([P, M2], wdt, tag=f"QT{b}", name=f"QT{b}") for b in range(NB)]
            KT = [pp.tile([P, M2], wdt, tag=f"KT{b}", name=f"KT{b}") for b in range(NB)]
            with tc.tile_pool(name="wcol", bufs=3) as wp, \
                 tc.tile_pool(name="qk_psum", bufs=1, space="PSUM") as qkp:
                # contiguous row-chunk loads: Wq[hi*P:(hi+1)*P, :] IS the lhsT
                # layout for (hi, all ho).  All 8 QT / 8 KT accumulators live
                # in PSUM simultaneously, packed two-per-bank.
                qt_pk = [qkp.tile([P, 2 * M2], F32, tag=f"qtp{i}", name=f"qtp{i}")
                         for i in range(NB // 2)]
                kt_pk = [qkp.tile([P, 2 * M2], F32, tag=f"ktp{i}", name=f"ktp{i}")
                         for i in range(NB // 2)]

                def qt_slice(ho):
                    return qt_pk[ho // 2][:, (ho % 2) * M2:(ho % 2 + 1) * M2]

                def kt_slice(ho):
                    return kt_pk[ho // 2][:, (ho % 2) * M2:(ho % 2 + 1) * M2]

                # KT = b_attn (rank-1) first: one accumulation group per bank
                for ho in range(NB):
                    nc.tensor.matmul(
                        out=kt_slice(ho),
                        lhsT=battn_sb[0:1, ho * P:(ho + 1) * P],
                        rhs=ones_sb[0:1, :M2],
                        start=(ho % 2 == 0), stop=False,
                    )
                for hi in range(NB):
                    wq_row = wp.tile([P, H], wdt, tag="wq", name="wq")
                    nc.sync.dma_start(
                        out=wq_row[:, :], in_=Wq[hi * P:(hi + 1) * P, :]
                    )
                    uk_row = wp.tile([P, H], wdt, tag="uk", name="uk")
                    nc.sync.dma_start(
                        out=uk_row[:, :], in_=Uk[hi * P:(hi + 1) * P, :]
                    )
                    for ho in range(NB):
                        nc.tensor.matmul(
                            out=qt_slice(ho),
                            lhsT=wq_row[:, ho * P:(ho + 1) * P],
                            rhs=clT[hi][:, :],
                            start=(hi == 0 and ho % 2 == 0),
                            stop=(hi == NB - 1 and ho % 2 == 1),
                        )
                    for ho in range(NB):
                        nc.tensor.matmul(
                            out=kt_slice(ho),
                            lhsT=uk_row[:, ho * P:(ho + 1) * P],
                            rhs=clT[hi][:, :],
                            start=False,
                            stop=(hi == NB - 1 and ho % 2 == 1),
                        )
                for ho in range(NB):
                    nc.vector.tensor_copy(out=QT[ho][:, :], in_=qt_slice(ho))
                    nc.scalar.activation(out=KT[ho][:, :], in_=kt_slice(ho), func=AF.Copy)

            # ---- pair scores: sc[e][(k,j,i)] = sum_h v[h]*tanh(q_i+k_j+b)
            NCH = (PAIRS_E + 511) // 512  # 512-col psum chunks per example
            # opened in reverse so closing e=0 first honors LIFO pool order
            scp_list = [tc.tile_pool(name=f"sc_psum{e}", bufs=1, space="PSUM")
                        for e in range(BL)]
            scp_objs = [None] * BL
            for e in reversed(range(BL)):
                scp_objs[e] = scp_list[e].__enter__()
            with tc.tile_pool(name="epool", bufs=2) as ep:
                sc_p = [scp_objs[e].tile([1, PAIRS_E], F32, tag=f"sc{e}", name=f"sc{e}")
                        for e in range(BL)]
                for hb in range(NB):
                    for e in range(BL):
                        e_in = ep.tile([P, PAIRS_E], wdt, tag="e_in", name="e_in", bufs=3)
                        kin = KT[hb][:, e * MT:(e + 1) * MT] \
                            .rearrange("p (k j) -> p k j", k=K) \
                            .unsqueeze(3).to_broadcast([P, K, C, C])
                        qin = QT[hb][:, e * MT:(e + 1) * MT] \
                            .rearrange("p (k i) -> p k i", k=K) \
                            .unsqueeze(2).to_broadcast([P, K, C, C])
                        eng = nc.vector if e == 0 else nc.gpsimd
                        eng.tensor_tensor(
                            out=e_in[:, :].rearrange("p (k j i) -> p k j i", k=K, j=C),
                            in0=kin, in1=qin, op=ALU.add,
                        )
                        e_t = ep.tile([P, PAIRS_E], wdt, tag="e_t", name="e_t", bufs=3)
                        nc.scalar.activation(out=e_t[:, :], in_=e_in[:, :], func=AF.Tanh)
                        for ch in range(NCH):
                            c0, c1 = ch * 512, min((ch + 1) * 512, PAIRS_E)
                            nc.tensor.matmul(
                                out=sc_p[e][0:1, c0:c1],
                                lhsT=vT_sb[:, hb:hb + 1],
                                rhs=e_t[:, c0:c1],
                                start=(hb == 0), stop=(hb == NB - 1),
                            )

                # ---- softmax over j; alpha laid out (k, j, i)
                abd = [pp.tile([MT, MT], wdt, tag=f"abd{e}", name=f"abd{e}") for e in range(BL)]
                for e in range(BL):
                    # exp written strided into the pre-zeroed row image of the
                    # block-diagonal alphaT matrix: position of value (k,j,i)
                    # is (k*C+j)*MT + k*C + i (affine in (k,j,i)).  After the
                    # in-place normalize, one DMA reshapes the row image to
                    # the [MT, MT] tile, so the ctx matmul depends on a
                    # single DMA (HW sync-wait limit).
                    p0 = list(list(diag[e][0:1, :].ap)[0])
                    dg_kji = diag[e][0:1, :].copy()
                    dg_kji.ap = _vec_pairs([p0, [C * MT + C, K], [MT, C], [1, C]])
                    dg_kij = diag[e][0:1, :].copy()
                    dg_kij.ap = _vec_pairs([p0, [C * MT + C, K], [1, C], [MT, C]])
                    nc.scalar.activation(
                        out=dg_kji, in_=sc_p[e][0:1, :].rearrange(
                            "p (k j i) -> p k j i", k=K, j=C),
                        func=AF.Exp,
                    )
                    sum_sb = pp.tile([1, K * C], F32, tag=f"sum{e}", name=f"sumsb{e}")
                    nc.vector.tensor_reduce(
                        out=sum_sb[:, :], in_=dg_kij, axis=AX.X, op=ALU.add,
                    )
                    rs_sb = pp.tile([1, K * C], F32, tag=f"rs{e}", name=f"rssb{e}")
                    nc.vector.reciprocal(out=rs_sb[:, :], in_=sum_sb[:, :])
                    nc.vector.tensor_tensor(
                        out=dg_kji, in0=dg_kji,
                        in1=rs_sb[:, :].rearrange("p (k i) -> p k i", k=K)
                            .unsqueeze(2).to_broadcast([1, K, C, C]),
                        op=ALU.mult,
                    )
                    nc.sync.dma_start(out=abd[e][:, :], in_=diag[e][0:1, :])
                    scp_list[e].__exit__(None, None, None)

            # ---- back half: ctx + attn + mrg with coexisting PSUM pools
            # (2 + 2 + 4 banks) and fully SBUF-resident weights -- no pool or
            # DMA serialization between the phases.
            ctxT = [pp.tile([P, M2], wdt, tag=f"ctxT{b}", name=f"ctxT{b}") for b in range(NB)]
            attnT = [pp.tile([P, M2], wdt, tag=f"attnT{b}", name=f"attnT{b}") for b in range(NB)]
            mrg_sb = [pp.tile([MT, H], F32, tag=f"mrg{e}", name=f"mrg{e}") for e in range(BL)]
            ND2 = H // 512 if H >= 512 else 1
            DW = min(H, 512)
            f_order = list(range(NB, 2 * NB)) + list(range(NB))
            with tc.tile_pool(name="at_psum", bufs=1, space="PSUM") as atp, \
                 tc.tile_pool(name="ctx_psum", bufs=2, space="PSUM") as cxp, \
                 tc.tile_pool(name="mg_psum", bufs=1, space="PSUM") as mgp:
                mg_p = [mgp.tile([MT, H], F32, tag=f"mg{e}", name=f"mg{e}") for e in range(BL)]
                for e in range(BL):
                    for d2 in range(ND2):
                        nc.tensor.matmul(
                            out=mg_p[e][:, d2 * DW:(d2 + 1) * DW],
                            lhsT=ones_sb[0:1, :MT],
                            rhs=bmrg_sb[0:1, d2 * DW:(d2 + 1) * DW],
                            start=True, stop=False,
                        )
                # mrg clT half: ready as soon as the pools open
                for f in range(NB):
                    for e in range(BL):
                        for d2 in range(ND2):
                            nc.tensor.matmul(
                                out=mg_p[e][:, d2 * DW:(d2 + 1) * DW],
                                lhsT=clT[f][:, e * MT:(e + 1) * MT],
                                rhs=wmrg_blk(f)[:, d2 * DW:(d2 + 1) * DW],
                                start=False, stop=False,
                            )

                # ctx matmuls (gated on abd by deps; the scheduler fills the
                # wait with the ready clT-half attn/mrg matmuls below)
                for e in range(BL):
                    for b in range(NB):
                        cx = cxp.tile([P, MT], F32, tag="cx", name="cx")
                        nc.tensor.matmul(
                            out=cx[:, :],
                            lhsT=cl_bf[e][:, b * P:(b + 1) * P],
                            rhs=abd[e][:, :],
                            start=True, stop=True,
                        )
                        nc.vector.tensor_copy(
                            out=ctxT[b][:, e * MT:(e + 1) * MT], in_=cx[:, :]
                        )

                # attn accumulation in two d-halves of 2 banks each (one
                # accumulation group per bank; even-d bias matmul starts it)
                for dh in range(2):
                    ds = [dh * (NB // 2) + i for i in range(NB // 2)]
                    ntile = (len(ds) + 1) // 2
                    at_p = [atp.tile([P, 2 * M2], F32, tag=f"at{i}", name=f"at{i}")
                            for i in range(ntile)]

                    def at_slice(d):
                        r = d - ds[0]
                        return at_p[r // 2][:, (r % 2) * M2:(r % 2 + 1) * M2]

                    def is_last_in_bank(d):
                        r = d - ds[0]
                        return r % 2 == 1 or r == len(ds) - 1

                    for d in ds:
                        nc.tensor.matmul(
                            out=at_slice(d),
                            lhsT=bout_sb[0:1, d * P:(d + 1) * P],
                            rhs=ones_sb[0:1, :M2],
                            start=((d - ds[0]) % 2 == 0), stop=False,
                        )
                    for fi, f in enumerate(f_order):
                        rhs_blk = ctxT[f] if f < NB else clT[f - NB]
                        for d in ds:
                            nc.tensor.matmul(
                                out=at_slice(d),
                                lhsT=wout_blk(f)[:, d * P:(d + 1) * P],
                                rhs=rhs_blk[:, :],
                                start=False,
                                stop=(fi == 2 * NB - 1 and is_last_in_bank(d)),
                            )
                    for d in ds:
                        nc.vector.tensor_copy(out=attnT[d][:, :], in_=at_slice(d))


                # mrg attnT half
                for f in range(NB, 2 * NB):
                    for e in range(BL):
                        for d2 in range(ND2):
                            nc.tensor.matmul(
                                out=mg_p[e][:, d2 * DW:(d2 + 1) * DW],
                                lhsT=attnT[f - NB][:, e * MT:(e + 1) * MT],
                                rhs=wmrg_blk(f)[:, d2 * DW:(d2 + 1) * DW],
                                start=False, stop=(f == 2 * NB - 1),
                            )
                for e in range(BL):
                    nc.scalar.activation(
                        out=mrg_sb[e][:, :], in_=mg_p[e][:, :], func=AF.Tanh
                    )

            # ---- scatter merged rows into out (after passthrough copy: WAW)
            for e in range(BL):
                nc.gpsimd.indirect_dma_start(
                    out=out2d,
                    out_offset=IndirectOffsetOnAxis(ap=idx_sb[:, e:e + 1], axis=0),
                    in_=mrg_sb[e][:, :],
                    in_offset=None,
                )

    return nc


# ---------------------------------------------------------------------------

S, B, H, K, C = 1024, 16, 1024, 8, 16
N_CORES = 8
BL = B // N_CORES
WEIGHT_DTYPE = mybir.dt.bfloat16  # F32 for exact; bf16 halves weight HBM + 2x PE

_prog_cache = {}


def _np_wdt():
    return mybir.dt.np(WEIGHT_DTYPE)


def _get_program():
    key = (S, BL, H, K, C, WEIGHT_DTYPE)
    if key not in _prog_cache:
        nc = build_program(S, BL, H, K, C, wdt=WEIGHT_DTYPE)
        nc.finalize()  # Bacc.finalize: wait-splitting, reg alloc, codegen
        _prog_cache[key] = nc
    return _prog_cache[key]


def make_in_maps(m_bank, coref_posi, Wq, Uk, b_attn, v_attn, W_out, b_out,
                 W_mrg, b_mrg):
    MT = K * C
    m_bank = np.ascontiguousarray(m_bank, dtype=np.float32)
    in_maps = []
    for c in range(N_CORES):
        mb_c = np.ascontiguousarray(m_bank[:, c * BL:(c + 1) * BL, :])
        # idx[m, e]: row of mention m of local example e in the (S*BL, H) view
        idx_c = np.empty((MT, BL), dtype=np.int32)
        for e in range(BL):
            pos = np.asarray(coref_posi[c * BL + e], dtype=np.int64).reshape(MT)
            idx_c[:, e] = (pos * BL + e).astype(np.int32)
        in_maps.append({
            "mb": mb_c,
            "idx": idx_c,
            "Wq": np.ascontiguousarray(Wq, dtype=_np_wdt()),
            "Uk": np.ascontiguousarray(Uk, dtype=_np_wdt()),
            "vattn": np.ascontiguousarray(v_attn, dtype=_np_wdt()).reshape(H),
            "battn": np.ascontiguousarray(b_attn, dtype=_np_wdt()).reshape(1, H),
            "Wout": np.ascontiguousarray(W_out, dtype=_np_wdt()),
            "bout": np.ascontiguousarray(b_out, dtype=_np_wdt()).reshape(1, H),
            "Wmrg": np.ascontiguousarray(W_mrg, dtype=_np_wdt()),
            "bmrg": np.ascontiguousarray(b_mrg, dtype=_np_wdt()).reshape(1, H),
        })
    return in_maps


def run(in_maps, trace=False, tmpdir=None):
    from concourse.bass_utils import run_bass_kernel_spmd
    nc = _get_program()
    return run_bass_kernel_spmd(
        nc, in_maps, list(range(N_CORES)), trace=trace, tmpdir=tmpdir
    )


def kernel(**inputs):
    in_maps = make_in_maps(**inputs)
    res = run(in_maps)
    outs = [res.results[c]["out"] for c in range(N_CORES)]
    return np.concatenate(outs, axis=1).astype(np.float32)


if __name__ == "__main__":
    nc = build_program()
    print("program built ok; instructions:",
          sum(len(bb.instructions) for f in nc.m.functions for bb in f.basicblocks)
          if hasattr(nc.m.functions[0], "basicblocks") else "n/a")
